# revision 4
# baseline (speedup 1.0000x reference)
"""Trainium2 Bass kernel for nn_CategoricalActivation (8-core data-parallel).

Reference semantics (per element x[s, b, h], column col=(b, h)):
    ss = x / (1 + |x|)                            # softsign
    boundaries b_c = x_raw[ind[c, col], col]      # 4 sampled rows per column
    counts = #{c : x > b_c} - 2.5
    cat  = cat_u[col] < 0.1
    ord  = (ord_u[col] < 0.7) & cat
    out  = ord ? 0.0 : (cat ? counts : ss)
(The "randomize_classes" remap is identically zero: counts values
{-2.5..1.5} never equal a class id 0..4, so remapped == 0 at ord cols.)

v4 design (per core; HBM-byte- and instruction-count-minimized):
  - ALL categorical columns (cat, ~10%) are compacted OUT of the bulk on
    the host: bulk is [S, KEEP=1872] bf16 (keep = non-cat columns only,
    padded).  Down from [S, 2048]: 8.6% fewer bulk bytes each way.  The
    softsign outputs for keep columns come back bf16; the host scatters
    them into the full output, writes 0.0 at ord columns and counts at
    catno (cat & ~ord) columns.
  - The bulk is staged in DEVICE TILE ORDER on the host: [4, 128, 7488]
    where granule g, partition p, (j*KEEP+w) maps to x row 512g+128j+p.
    Each in/out DMA is one fully contiguous 1.92 MB transfer (4 in + 4
    out DMAs total, vs 33 chunk DMAs in v3) - near line-rate DMA and ~3x
    fewer instructions/semaphores, which also shrinks the fixed
    preamble/teardown overhead (instruction-stream load + end-of-program
    semaphore drain chains measured ~12us in the v3 trace).
  - Compute granule is half an in-tile [128, 3744]: |x| on the DVE
    (bitwise_and 0x7FFF on an int16 view), r = 1/(1+|x|) on the Scalar
    engine (bias=1.0 folded into the activation), out = x*r in-place on
    the DVE.  Scalar-engine stream (~30us) stays off the critical path
    because loads arrive progressively (4 separate in DMAs).
  - counts compare RAW f32 values (order-equivalent to comparing
    softsign values; bf16 would create compare ties).  Host stages
    xce[KMAX=68, 4+S]: per catno column its 4 boundary values then the
    raw column.  4 fused compare passes on the DVE slack produce
    cnt = #{c: x > b_c} in {0..4}, returned bf16 (exact); host
    subtracts 2.5 while unsharding.
"""

import numpy as np

S = 2048
B = 16
H = 1024
NCORES = 8
BLOC = B // NCORES         # 2
C = BLOC * H               # 2048 columns per core
P = 128
KEEP = 1872                # padded non-cat (bulk) column slots per core
G = 4                      # in/out DMA granules (512 rows each)
JG = 4                     # row chunks per granule
W4 = JG * KEEP             # 7488 free elements per in-tile
W2 = W4 // 2               # 3744: compute granule (half tile)
KMAX = 68                  # padded catno (counts) column slots per core
NC5 = 5

_CACHE = {}


def _split_multi_waits(nc, scr_ap=None, max_waits=1):
    """This container's walrus rejects >1 sync-wait per instruction; hoist
    extra waits onto cheap same-engine carrier instructions inserted just
    before (tiny Memset on the pipelined engines - a Drain there would
    flush the pipe at ~0.4-2.4us - and Drain on the sequencer-only ones)."""
    import concourse.mybir as mybir

    memset_engines = {mybir.EngineType.DVE, mybir.EngineType.Pool}
    n_split = 0
    for f in nc.m.functions:
        for blk in f.blocks:
            insts = blk.instructions
            i = 0
            while i < len(insts):
                ins = insts[i]
                si = ins.sync_info
                if si is not None and len(si.on_wait) > max_waits:
                    waits = list(si.on_wait)
                    keep = waits[-max_waits:]
                    hoist = waits[:-max_waits]
                    for w in hoist:
                        if scr_ap is not None and ins.engine in memset_engines:
                            d = mybir.InstMemset(
                                name=f"I-{nc.next_id()}", mode="Const",
                                ins=[], outs=[scr_ap], constant=0)
                        else:
                            d = mybir.InstDrain(
                                name=f"I-{nc.next_id()}", ins=[], outs=[],
                                bass_is_fusable=False)
                        d.engine = ins.engine
                        d.sync_info = mybir.SyncInfo(on_wait=[w], on_update=[])
                        insts.insert(i, d)
                        i += 1
                        n_split += 1
                    si.on_wait = keep
                    ins.sync_info = si
                i += 1
    return n_split


def _act_unary(nc, out_ap, in_ap, func, bias=0.0):
    """One scalar-engine activation, float-immediate bias (bypasses the
    bass wrapper so Reciprocal is allowed; HW-measured ~1.2e-5 max err)."""
    import concourse.mybir as mybir

    eng = nc.scalar
    ins_ = [
        eng.lower_ap(in_ap),
        mybir.ImmediateValue(dtype=mybir.dt.float32, value=float(bias)),
        mybir.ImmediateValue(dtype=mybir.dt.float32, value=1.0),
        mybir.ImmediateValue(dtype=mybir.dt.float32, value=0.0),
    ]
    return eng.add_instruction(
        mybir.InstActivation(
            name=nc.get_next_instruction_name(),
            func=func,
            ins=ins_,
            outs=[eng.lower_ap(out_ap)],
        )
    )


def _build_program():
    import contextlib

    import concourse.bass as bass
    import concourse.tile as tile
    from concourse import mybir

    A = mybir.AluOpType
    F = mybir.ActivationFunctionType
    f32 = mybir.dt.float32
    bf16 = mybir.dt.bfloat16
    i16 = mybir.dt.int16
    i32 = mybir.dt.int32

    nc = bass.Bass()
    x_in = nc.dram_tensor("x", [G, P, W4], bf16, kind="ExternalInput")
    xce_in = nc.dram_tensor("xce", [KMAX, 4 + S], f32, kind="ExternalInput")
    out_d = nc.dram_tensor("out", [G, P, W4], bf16, kind="ExternalOutput")
    cnt_d = nc.dram_tensor("cnt", [KMAX, S], bf16, kind="ExternalOutput")

    with tile.TileContext(nc) as tc:
        with contextlib.ExitStack() as ctx:
            singles = ctx.enter_context(tc.tile_pool(name="singles", bufs=1))
            xp = ctx.enter_context(tc.tile_pool(name="xp", bufs=G))
            ap_ = ctx.enter_context(tc.tile_pool(name="ap", bufs=2))
            rp = ctx.enter_context(tc.tile_pool(name="rp", bufs=2))

            scr = singles.tile([1, 8], i32, name="scr")
            nc.vector.memset(scr, 0)

            # xce rides FIRST on the single HWDGE FIFO ring: the count
            # passes the scheduler hoists into early DVE slack must never
            # wait on it (a late xce serializes the whole DVE stream and
            # compute-paces the out DMAs - v4 lost 8us to exactly that)
            xce = singles.tile([KMAX, 4 + S], f32)
            nc.sync.dma_start(out=xce, in_=xce_in[:, :])

            # then every bulk in-DMA upfront: input streams at line rate,
            # compute never gates a load
            xts = []
            for g in range(G):
                xt = xp.tile([P, W4], bf16, tag="xt", name=f"xt{g}")
                nc.sync.dma_start(out=xt, in_=x_in[g, :, :])
                xts.append(xt)

            cnt = singles.tile([KMAX, S], f32)
            cntb = singles.tile([KMAX, S], bf16)

            def count_pass(c):
                # one boundary compare over the catno columns (DVE slack)
                if c == 0:
                    nc.vector.tensor_scalar(
                        out=cnt, in0=xce[:, 4:], scalar1=xce[:, 0:1],
                        scalar2=None, op0=A.is_gt)
                else:
                    nc.vector.scalar_tensor_tensor(
                        out=(cntb if c == 3 else cnt), in0=xce[:, 4:],
                        scalar=xce[:, c:c + 1], in1=cnt,
                        op0=A.is_gt, op1=A.add)

            for cg in range(2 * G):
                g, h = divmod(cg, 2)
                xv = xts[g][:, h * W2:(h + 1) * W2]
                xvi = xts[g].bitcast(i16)[:, h * W2:(h + 1) * W2]
                absx = ap_.tile([P, W2], bf16, tag="absx", name="absx")
                nc.vector.tensor_scalar(out=absx.bitcast(i16),
                                        in0=xvi,
                                        scalar1=0x7FFF, scalar2=None,
                                        op0=A.bitwise_and)
                ract = rp.tile([P, W2], bf16, tag="ract", name="ract")
                _act_unary(nc, ract[:, :], absx[:, :], F.Reciprocal, bias=1.0)
                nc.vector.tensor_tensor(out=xv, in0=xv, in1=ract, op=A.mult)
                if h == 1:
                    # whole granule computed -> one contiguous 1.9MB store
                    nc.sync.dma_start(out=out_d[g, :, :], in_=xts[g])
                if 1 <= cg <= 4:
                    count_pass(cg - 1)
                    if cg == 4:
                        nc.sync.dma_start(out=cnt_d[:, :], in_=cntb)

    _split_multi_waits(nc, scr_ap=nc.vector.lower_ap(scr[0:1, 0:1]))
    return nc


def _stage_bulk(xk):
    """[S, KEEP] f32 -> device tile order [G, P, JG*KEEP] bf16
    (granule g, partition p, segment j: row 512g + 128j + p)."""
    import ml_dtypes
    v = xk.reshape(G, JG, P, KEEP).transpose(0, 2, 1, 3).reshape(G, P, W4)
    return np.ascontiguousarray(v).astype(ml_dtypes.bfloat16)


def _unstage_bulk(ob):
    """[G, P, JG*KEEP] bf16 -> [S, KEEP] f32."""
    v = np.asarray(ob).astype(np.float32)
    return v.reshape(G, P, JG, KEEP).transpose(0, 2, 1, 3).reshape(S, KEEP)


def kernel(x, ind, cat_u, ord_u, perm, num_classes):
    from concourse.bass_utils import run_bass_kernel_spmd

    assert int(num_classes) == NC5
    x = np.ascontiguousarray(x, dtype=np.float32)
    ind = np.ascontiguousarray(ind, dtype=np.int32)
    cat_u = np.asarray(cat_u, dtype=np.float32)
    ord_u = np.asarray(ord_u, dtype=np.float32)
    assert x.shape == (S, B, H) and ind.shape == (4, B, H)

    cat = cat_u < np.float32(0.1)
    ordm = (ord_u < np.float32(0.7)) & cat
    catno = cat & ~ordm
    in_maps = []
    keep_lists = []
    cat_lists = []
    for m in range(NCORES):
        bs = slice(BLOC * m, BLOC * (m + 1))
        xm = x[:, bs, :].reshape(S, C)
        indm = ind[:, bs, :].reshape(4, C)
        kcols = np.nonzero(~cat[bs].reshape(C))[0].astype(np.int32)
        ccols = np.nonzero(catno[bs].reshape(C))[0].astype(np.int32)
        nk, kc = len(kcols), len(ccols)
        assert nk <= KEEP, f"core {m}: {nk} keep columns exceed KEEP"
        assert kc <= KMAX, f"core {m}: {kc} catno columns exceed KMAX"
        keep_lists.append(kcols)
        cat_lists.append(ccols)
        xk = np.zeros((S, KEEP), np.float32)
        xk[:, :nk] = xm[:, kcols]
        xce = np.zeros((KMAX, 4 + S), np.float32)
        xce[:kc, 4:] = xm[:, ccols].T
        xce[:kc, 0:4] = xm[indm[:, ccols], ccols].T
        in_maps.append({"x": _stage_bulk(xk), "xce": xce})

    if "nc" not in _CACHE:
        _CACHE["nc"] = _build_program()
    res = run_bass_kernel_spmd(_CACHE["nc"], in_maps,
                               core_ids=list(range(NCORES)))
    out = np.empty((S, B, H), np.float32)
    for m in range(NCORES):
        bs = slice(BLOC * m, BLOC * (m + 1))
        om = np.zeros((S, C), np.float32)
        kcols, ccols = keep_lists[m], cat_lists[m]
        ok = _unstage_bulk(res.results[m]["out"])
        om[:, kcols] = ok[:, :len(kcols)]
        if len(ccols):
            cm = np.asarray(res.results[m]["cnt"][:len(ccols)])
            om[:, ccols] = cm.astype(np.float32).T - np.float32(2.5)
        out[:, bs, :] = om.reshape(S, BLOC, H)
    return out


# revision 6
# speedup vs baseline: 1.1713x; 1.1713x over previous
"""Trainium2 Bass kernel for nn_CategoricalActivation (8-core data-parallel).

Reference semantics (per element x[s, b, h], column col=(b, h)):
    ss = x / (1 + |x|)                            # softsign
    boundaries b_c = x_raw[ind[c, col], col]      # 4 sampled rows per column
    counts = #{c : x > b_c} - 2.5
    cat  = cat_u[col] < 0.1
    ord  = (ord_u[col] < 0.7) & cat
    out  = ord ? 0.0 : (cat ? counts : ss)
(The "randomize_classes" remap is identically zero: counts values
{-2.5..1.5} never equal a class id 0..4, so remapped == 0 at ord cols.)

v6 design (per core):
  - ALL categorical columns (cat, ~10%) are compacted OUT of the bulk on
    the host: bulk is [S, KEEP=1872] bf16 (non-cat columns, padded) -
    8.6% fewer bulk bytes each way than [S, 2048].  Host scatters the
    softsign results into the full output, writes 0.0 at ord columns and
    counts at catno columns.
  - Bulk staged in DEVICE TILE ORDER: [TCH=16, 128, 1872], chunk t row r
    = x row 128t+r, so every chunk DMA is one contiguous 479 KB block.
  - Chunk DMAs stay ~0.5 MB ON PURPOSE: DMAs outstanding on the HWDGE
    ring progress CONCURRENTLY (fair packet round-robin, not FIFO), so
    a few large upfront loads all complete clustered at the end of the
    read stream and compute starts ~25us late (v4/v5 lost 8-15us to
    this).  With 16 chunk loads the first chunks complete early and the
    softsign pipeline (DVE abs -> ACT recip -> DVE mult -> store) runs
    behind the read stream.
  - counts side-channel is bf16: xce[KMAX=68, 4+S] holds per catno
    column its 4 boundary values then the raw column, all bf16.  Device
    computes cnt = #{c: x_bf16 > b_bf16} in {0..4} (4 fused DVE compare
    passes in stream slack).  bf16 rounding is monotone, so the device
    count differs from the f32 count only where bf16(x) == bf16(b); the
    host adds that correction (tie & (x > b), pure numpy) while
    unsharding, then subtracts 2.5.  xce rides FIRST on the ring so the
    count passes (which the scheduler front-loads into DVE slack) never
    stall the DVE stream.
"""

import numpy as np

S = 2048
B = 16
H = 1024
NCORES = 8
BLOC = B // NCORES         # 2
C = BLOC * H               # 2048 columns per core
P = 128
KEEP = 1872                # padded non-cat (bulk) column slots per core
TCH = S // P               # 16 row chunks
KMAX = 68                  # padded catno (counts) column slots per core
NC5 = 5

_CACHE = {}


def _split_multi_waits(nc, scr_ap=None, max_waits=1):
    """This container's walrus rejects >1 sync-wait per instruction; hoist
    extra waits onto cheap same-engine carrier instructions inserted just
    before (tiny Memset on the pipelined engines - a Drain there would
    flush the pipe at ~0.4-2.4us - and Drain on the sequencer-only ones)."""
    import concourse.mybir as mybir

    memset_engines = {mybir.EngineType.DVE, mybir.EngineType.Pool}
    n_split = 0
    for f in nc.m.functions:
        for blk in f.blocks:
            insts = blk.instructions
            i = 0
            while i < len(insts):
                ins = insts[i]
                si = ins.sync_info
                if si is not None and len(si.on_wait) > max_waits:
                    waits = list(si.on_wait)
                    keep = waits[-max_waits:]
                    hoist = waits[:-max_waits]
                    for w in hoist:
                        if scr_ap is not None and ins.engine in memset_engines:
                            d = mybir.InstMemset(
                                name=f"I-{nc.next_id()}", mode="Const",
                                ins=[], outs=[scr_ap], constant=0)
                        else:
                            d = mybir.InstDrain(
                                name=f"I-{nc.next_id()}", ins=[], outs=[],
                                bass_is_fusable=False)
                        d.engine = ins.engine
                        d.sync_info = mybir.SyncInfo(on_wait=[w], on_update=[])
                        insts.insert(i, d)
                        i += 1
                        n_split += 1
                    si.on_wait = keep
                    ins.sync_info = si
                i += 1
    return n_split


def _act_unary(nc, out_ap, in_ap, func, bias=0.0):
    """One scalar-engine activation, float-immediate bias (bypasses the
    bass wrapper so Reciprocal is allowed; HW-measured ~1.2e-5 max err)."""
    import concourse.mybir as mybir

    eng = nc.scalar
    ins_ = [
        eng.lower_ap(in_ap),
        mybir.ImmediateValue(dtype=mybir.dt.float32, value=float(bias)),
        mybir.ImmediateValue(dtype=mybir.dt.float32, value=1.0),
        mybir.ImmediateValue(dtype=mybir.dt.float32, value=0.0),
    ]
    return eng.add_instruction(
        mybir.InstActivation(
            name=nc.get_next_instruction_name(),
            func=func,
            ins=ins_,
            outs=[eng.lower_ap(out_ap)],
        )
    )


def _build_program():
    import contextlib

    import concourse.bass as bass
    import concourse.tile as tile
    from concourse import mybir

    A = mybir.AluOpType
    F = mybir.ActivationFunctionType
    f32 = mybir.dt.float32
    bf16 = mybir.dt.bfloat16
    i16 = mybir.dt.int16
    i32 = mybir.dt.int32

    nc = bass.Bass()
    x_in = nc.dram_tensor("x", [TCH, P, KEEP], bf16, kind="ExternalInput")
    xce_in = nc.dram_tensor("xce", [KMAX, S], bf16, kind="ExternalInput")
    bval_in = nc.dram_tensor("bval", [KMAX, 4], f32, kind="ExternalInput")
    out_d = nc.dram_tensor("out", [TCH, P, KEEP], bf16, kind="ExternalOutput")
    cnt_d = nc.dram_tensor("cnt", [KMAX, S], bf16, kind="ExternalOutput")

    with tile.TileContext(nc) as tc:
        with contextlib.ExitStack() as ctx:
            singles = ctx.enter_context(tc.tile_pool(name="singles", bufs=1))
            xp = ctx.enter_context(tc.tile_pool(name="xp", bufs=TCH))
            up = ctx.enter_context(tc.tile_pool(name="up", bufs=6))

            scr = singles.tile([1, 8], i32, name="scr")
            nc.vector.memset(scr, 0)

            # xce rides FIRST on the single HWDGE FIFO ring: the count
            # passes the scheduler hoists into early DVE slack must never
            # wait on a late load (that serializes the whole DVE stream)
            bval = singles.tile([KMAX, 4], f32)
            nc.sync.dma_start(out=bval, in_=bval_in[:, :])
            xce = singles.tile([KMAX, S], bf16)
            nc.sync.dma_start(out=xce, in_=xce_in[:, :])

            # every chunk in-DMA upfront, one SBUF slot each: loads never
            # wait on buffer recycling and the early chunks complete early
            # (ring shares bandwidth round-robin among outstanding DMAs)
            xts = []
            for t in range(TCH):
                xt = xp.tile([P, KEEP], bf16, tag="xt", name=f"xt{t}")
                nc.sync.dma_start(out=xt, in_=x_in[t, :, :])
                xts.append(xt)

            cnt = singles.tile([KMAX, S], bf16)
            cntb = singles.tile([KMAX, S], bf16)

            def count_pass(c):
                # one bf16 boundary compare over the catno columns
                if c == 0:
                    nc.vector.tensor_scalar(
                        out=cnt, in0=xce, scalar1=bval[:, 0:1],
                        scalar2=None, op0=A.is_gt)
                else:
                    nc.vector.scalar_tensor_tensor(
                        out=(cntb if c == 3 else cnt), in0=xce,
                        scalar=bval[:, c:c + 1], in1=cnt,
                        op0=A.is_gt, op1=A.add)

            for t in range(TCH):
                xt = xts[t]
                absx = up.tile([P, KEEP], bf16, tag="absx", name="absx")
                nc.vector.tensor_scalar(out=absx.bitcast(i16),
                                        in0=xt.bitcast(i16),
                                        scalar1=0x7FFF, scalar2=None,
                                        op0=A.bitwise_and)
                ract = up.tile([P, KEEP], bf16, tag="ract", name="ract")
                _act_unary(nc, ract[:, :], absx[:, :], F.Reciprocal, bias=1.0)
                nc.vector.tensor_tensor(out=xt, in0=xt, in1=ract, op=A.mult)
                nc.sync.dma_start(out=out_d[t, :, :], in_=xt)
                if 1 <= t <= 4:
                    count_pass(t - 1)
                if t == 8:
                    # mid-stream: ring reaches it long after cp3 is done,
                    # and it does not extend the final out-DMA tail
                    nc.sync.dma_start(out=cnt_d[:, :], in_=cntb)

    _split_multi_waits(nc, scr_ap=nc.vector.lower_ap(scr[0:1, 0:1]))
    return nc


def _stage_bulk(xk):
    """[S, KEEP] f32 -> device chunk order [TCH, P, KEEP] bf16."""
    import ml_dtypes
    return np.ascontiguousarray(
        xk.reshape(TCH, P, KEEP)).astype(ml_dtypes.bfloat16)


def _unstage_bulk(ob):
    """[TCH, P, KEEP] bf16 -> [S, KEEP] f32."""
    return np.asarray(ob).astype(np.float32).reshape(S, KEEP)


def kernel(x, ind, cat_u, ord_u, perm, num_classes):
    import ml_dtypes
    from concourse.bass_utils import run_bass_kernel_spmd

    assert int(num_classes) == NC5
    x = np.ascontiguousarray(x, dtype=np.float32)
    ind = np.ascontiguousarray(ind, dtype=np.int32)
    cat_u = np.asarray(cat_u, dtype=np.float32)
    ord_u = np.asarray(ord_u, dtype=np.float32)
    assert x.shape == (S, B, H) and ind.shape == (4, B, H)

    cat = cat_u < np.float32(0.1)
    ordm = (ord_u < np.float32(0.7)) & cat
    catno = cat & ~ordm
    in_maps = []
    keep_lists = []
    cat_lists = []
    corr_lists = []
    for m in range(NCORES):
        bs = slice(BLOC * m, BLOC * (m + 1))
        xm = x[:, bs, :].reshape(S, C)
        indm = ind[:, bs, :].reshape(4, C)
        kcols = np.nonzero(~cat[bs].reshape(C))[0].astype(np.int32)
        ccols = np.nonzero(catno[bs].reshape(C))[0].astype(np.int32)
        nk, kc = len(kcols), len(ccols)
        assert nk <= KEEP, f"core {m}: {nk} keep columns exceed KEEP"
        assert kc <= KMAX, f"core {m}: {kc} catno columns exceed KMAX"
        keep_lists.append(kcols)
        cat_lists.append(ccols)
        xk = np.zeros((S, KEEP), np.float32)
        xk[:, :nk] = xm[:, kcols]
        v = xm[:, ccols].T                       # [kc, S] f32
        t_ = xm[indm[:, ccols], ccols].T         # [kc, 4] f32
        vb = v.astype(ml_dtypes.bfloat16)
        tb = t_.astype(ml_dtypes.bfloat16)
        # bf16 rounding is monotone: device bf16 count == f32 count except
        # at bf16 ties, where is_gt lost (x > b).  Correct on the host.
        corr = ((vb[:, None, :] == tb[:, :, None])
                & (v[:, None, :] > t_[:, :, None])).sum(1).astype(np.float32)
        corr_lists.append(corr)                  # [kc, S]
        xce = np.zeros((KMAX, S), ml_dtypes.bfloat16)
        xce[:kc] = vb
        bvf = np.zeros((KMAX, 4), np.float32)
        bvf[:kc] = tb.astype(np.float32)
        in_maps.append({"x": _stage_bulk(xk), "xce": xce, "bval": bvf})

    if "nc" not in _CACHE:
        _CACHE["nc"] = _build_program()
    res = run_bass_kernel_spmd(_CACHE["nc"], in_maps,
                               core_ids=list(range(NCORES)))
    out = np.empty((S, B, H), np.float32)
    for m in range(NCORES):
        bs = slice(BLOC * m, BLOC * (m + 1))
        om = np.zeros((S, C), np.float32)
        kcols, ccols = keep_lists[m], cat_lists[m]
        ok = _unstage_bulk(res.results[m]["out"])
        om[:, kcols] = ok[:, :len(kcols)]
        if len(ccols):
            cm = np.asarray(res.results[m]["cnt"][:len(ccols)]).astype(np.float32)
            om[:, ccols] = (cm + corr_lists[m]).T - np.float32(2.5)
        out[:, bs, :] = om.reshape(S, BLOC, H)
    return out


# revision 8
# speedup vs baseline: 1.2767x; 1.0900x over previous
"""Trainium2 Bass kernel for nn_CategoricalActivation (8-core data-parallel).

Reference semantics (per element x[s, b, h], column col=(b, h)):
    ss = x / (1 + |x|)                            # softsign
    boundaries b_c = x_raw[ind[c, col], col]      # 4 sampled rows per column
    counts = #{c : x > b_c} - 2.5
    cat  = cat_u[col] < 0.1
    ord  = (ord_u[col] < 0.7) & cat
    out  = ord ? 0.0 : (cat ? counts : ss)
(The "randomize_classes" remap is identically zero: counts values
{-2.5..1.5} never equal a class id 0..4, so remapped == 0 at ord cols.)

v7 design (per core):
  - Device does the bulk softsign stream; everything per-column/sparse
    (boundary gathers, counts for the ~3% catno columns, ord zeros,
    scatter) happens on the host while staging/unsharding.  Rationale
    from v6 tracing: side-channel tensors span only <=68 SBUF partitions,
    so their DMA descriptors pile onto a few DMA engines; the last bulk
    store's completion then trails the slowest engine by ~6.5us.  A pure
    [128, *] stream keeps all 16 engines perfectly balanced.
  - ALL categorical columns (~10%) are compacted OUT of the bulk on the
    host: bulk is [S, KEEP=1872] bf16 (non-cat columns, padded), staged
    in device chunk order [TCH=16, 128, 1872] so each chunk DMA is one
    contiguous 479 KB block.
  - Chunk DMAs stay ~0.5 MB: DMAs outstanding on the HWDGE ring progress
    CONCURRENTLY (fair packet-level round-robin, not FIFO), so a few
    large upfront loads all complete clustered at the read-stream end
    and compute starts ~20us late (v4/v5 lost 8-15us to this).  With 16
    chunk loads dispatched back-to-back the first chunk lands ~3us after
    dispatch and the softsign pipeline (DVE |x| -> ACT 1/(1+|x|) -> DVE
    mult -> store) runs just behind the read stream.
  - The first out-DMA is gated on the LAST in-DMA (extra semaphore wait
    patched in after scheduling): the ring then does one pure-read burst
    followed by one pure-write burst instead of packet-interleaving
    reads with writes mid-stream (HBM bus turnaround costs bandwidth).
"""

import numpy as np

S = 2048
B = 16
H = 1024
NCORES = 8
BLOC = B // NCORES         # 2
C = BLOC * H               # 2048 columns per core
P = 128
KEEP = 1872                # padded non-cat (bulk) column slots per core
TCH = S // P               # 16 row chunks
NC5 = 5
GATE_OUTS = True           # first store waits for last load (burst phases)

_CACHE = {}


def _split_multi_waits(nc, scr_ap=None, max_waits=1):
    """This container's walrus rejects >1 sync-wait per instruction; hoist
    extra waits onto cheap same-engine carrier instructions inserted just
    before (tiny Memset on the pipelined engines - a Drain there would
    flush the pipe at ~0.4-2.4us - and Drain on the sequencer-only ones)."""
    import concourse.mybir as mybir

    memset_engines = {mybir.EngineType.DVE, mybir.EngineType.Pool}
    n_split = 0
    for f in nc.m.functions:
        for blk in f.blocks:
            insts = blk.instructions
            i = 0
            while i < len(insts):
                ins = insts[i]
                si = ins.sync_info
                if si is not None and len(si.on_wait) > max_waits:
                    waits = list(si.on_wait)
                    keep = waits[-max_waits:]
                    hoist = waits[:-max_waits]
                    for w in hoist:
                        if scr_ap is not None and ins.engine in memset_engines:
                            d = mybir.InstMemset(
                                name=f"I-{nc.next_id()}", mode="Const",
                                ins=[], outs=[scr_ap], constant=0)
                        else:
                            d = mybir.InstDrain(
                                name=f"I-{nc.next_id()}", ins=[], outs=[],
                                bass_is_fusable=False)
                        d.engine = ins.engine
                        d.sync_info = mybir.SyncInfo(on_wait=[w], on_update=[])
                        insts.insert(i, d)
                        i += 1
                        n_split += 1
                    si.on_wait = keep
                    ins.sync_info = si
                i += 1
    return n_split


def _gate_outs_on_last_in(nc):
    """Insert one SP Drain before the first out-DMA waiting on the last
    in-DMA's completion semaphore: the ring finishes the read burst before
    any write descriptors queue behind it (avoids HBM read/write
    interleaving mid-stream).  Post-scheduling BIR patch."""
    import concourse.mybir as mybir

    for f in nc.m.functions:
        for blk in f.blocks:
            insts = blk.instructions
            last_in = None           # (sem_id, cumulative threshold)
            cum = {}
            first_out_idx = None
            for i, ins in enumerate(insts):
                if not isinstance(ins, mybir.InstDMACopy):
                    continue
                si = ins.sync_info
                upd = si.on_update[0] if si and si.on_update else None
                if upd is not None:
                    cum[upd.id] = cum.get(upd.id, 0) + upd.update_value
                src = ins.ins[0].memref if ins.ins else ""
                dst = ins.outs[0].memref if ins.outs else ""
                if src == "x" and upd is not None:
                    last_in = (upd.id, cum[upd.id], upd.ant_name)
                if dst == "out" and first_out_idx is None:
                    first_out_idx = i
            if last_in is None or first_out_idx is None:
                continue
            sem_id, thresh, ant = last_in
            w = mybir.SyncWait(sync_type="semaphore", id=sem_id,
                               ant_name=ant, wait_mode="sem-ge-imm",
                               wait_value=thresh)
            d = mybir.InstDrain(name=f"I-{nc.next_id()}", ins=[], outs=[],
                                bass_is_fusable=False)
            d.engine = insts[first_out_idx].engine
            d.sync_info = mybir.SyncInfo(on_wait=[w], on_update=[])
            insts.insert(first_out_idx, d)
            return True
    return False


def _act_unary(nc, out_ap, in_ap, func, bias=0.0):
    """One scalar-engine activation, float-immediate bias (bypasses the
    bass wrapper so Reciprocal is allowed; HW-measured ~1.2e-5 max err)."""
    import concourse.mybir as mybir

    eng = nc.scalar
    ins_ = [
        eng.lower_ap(in_ap),
        mybir.ImmediateValue(dtype=mybir.dt.float32, value=float(bias)),
        mybir.ImmediateValue(dtype=mybir.dt.float32, value=1.0),
        mybir.ImmediateValue(dtype=mybir.dt.float32, value=0.0),
    ]
    return eng.add_instruction(
        mybir.InstActivation(
            name=nc.get_next_instruction_name(),
            func=func,
            ins=ins_,
            outs=[eng.lower_ap(out_ap)],
        )
    )


def _build_program():
    import contextlib

    import concourse.bass as bass
    import concourse.tile as tile
    from concourse import mybir

    A = mybir.AluOpType
    F = mybir.ActivationFunctionType
    bf16 = mybir.dt.bfloat16
    i16 = mybir.dt.int16
    i32 = mybir.dt.int32

    nc = bass.Bass()
    x_in = nc.dram_tensor("x", [TCH, P, KEEP], bf16, kind="ExternalInput")
    out_d = nc.dram_tensor("out", [TCH, P, KEEP], bf16, kind="ExternalOutput")

    with tile.TileContext(nc) as tc:
        with contextlib.ExitStack() as ctx:
            singles = ctx.enter_context(tc.tile_pool(name="singles", bufs=1))
            xp = ctx.enter_context(tc.tile_pool(name="xp", bufs=TCH))
            up = ctx.enter_context(tc.tile_pool(name="up", bufs=6))

            scr = singles.tile([1, 8], i32, name="scr")
            nc.vector.memset(scr, 0)

            # every chunk in-DMA upfront, one SBUF slot each: loads never
            # wait on buffer recycling; chunk 0 completes ~3us after
            # dispatch (ring fair-share stays shallow early)
            xts = []
            for t in range(TCH):
                xt = xp.tile([P, KEEP], bf16, tag="xt", name=f"xt{t}")
                nc.sync.dma_start(out=xt, in_=x_in[t, :, :])
                xts.append(xt)

            for t in range(TCH):
                xt = xts[t]
                absx = up.tile([P, KEEP], bf16, tag="absx", name="absx")
                nc.vector.tensor_scalar(out=absx.bitcast(i16),
                                        in0=xt.bitcast(i16),
                                        scalar1=0x7FFF, scalar2=None,
                                        op0=A.bitwise_and)
                ract = up.tile([P, KEEP], bf16, tag="ract", name="ract")
                _act_unary(nc, ract[:, :], absx[:, :], F.Reciprocal, bias=1.0)
                nc.vector.tensor_tensor(out=xt, in0=xt, in1=ract, op=A.mult)
                nc.sync.dma_start(out=out_d[t, :, :], in_=xt)

    _split_multi_waits(nc, scr_ap=nc.vector.lower_ap(scr[0:1, 0:1]))
    if GATE_OUTS:
        _gate_outs_on_last_in(nc)
    return nc


def _stage_bulk(xk):
    """[S, KEEP] f32 -> device chunk order [TCH, P, KEEP] bf16."""
    import ml_dtypes
    return np.ascontiguousarray(
        xk.reshape(TCH, P, KEEP)).astype(ml_dtypes.bfloat16)


def _unstage_bulk(ob):
    """[TCH, P, KEEP] bf16 -> [S, KEEP] f32."""
    return np.asarray(ob).astype(np.float32).reshape(S, KEEP)


def kernel(x, ind, cat_u, ord_u, perm, num_classes):
    from concourse.bass_utils import run_bass_kernel_spmd

    assert int(num_classes) == NC5
    x = np.ascontiguousarray(x, dtype=np.float32)
    ind = np.ascontiguousarray(ind, dtype=np.int32)
    cat_u = np.asarray(cat_u, dtype=np.float32)
    ord_u = np.asarray(ord_u, dtype=np.float32)
    assert x.shape == (S, B, H) and ind.shape == (4, B, H)

    cat = cat_u < np.float32(0.1)
    ordm = (ord_u < np.float32(0.7)) & cat
    catno = cat & ~ordm
    in_maps = []
    keep_lists = []
    cat_lists = []
    cnt_lists = []
    for m in range(NCORES):
        bs = slice(BLOC * m, BLOC * (m + 1))
        xm = x[:, bs, :].reshape(S, C)
        indm = ind[:, bs, :].reshape(4, C)
        kcols = np.nonzero(~cat[bs].reshape(C))[0].astype(np.int32)
        ccols = np.nonzero(catno[bs].reshape(C))[0].astype(np.int32)
        nk = len(kcols)
        assert nk <= KEEP, f"core {m}: {nk} keep columns exceed KEEP"
        keep_lists.append(kcols)
        cat_lists.append(ccols)
        xk = np.zeros((S, KEEP), np.float32)
        xk[:, :nk] = xm[:, kcols]
        # counts for the ~3% catno columns: f32 compares, exactly the
        # reference ordering (softsign is strictly monotone)
        v = xm[:, ccols]                          # [S, kc]
        t_ = xm[indm[:, ccols], ccols]            # [4, kc]
        cnt_lists.append((v[None] > t_[:, None]).sum(0).astype(np.float32)
                         - np.float32(2.5))      # [S, kc]
        in_maps.append({"x": _stage_bulk(xk)})

    if "nc" not in _CACHE:
        _CACHE["nc"] = _build_program()
    res = run_bass_kernel_spmd(_CACHE["nc"], in_maps,
                               core_ids=list(range(NCORES)))
    out = np.empty((S, B, H), np.float32)
    for m in range(NCORES):
        bs = slice(BLOC * m, BLOC * (m + 1))
        om = np.zeros((S, C), np.float32)
        kcols, ccols = keep_lists[m], cat_lists[m]
        ok = _unstage_bulk(res.results[m]["out"])
        om[:, kcols] = ok[:, :len(kcols)]
        if len(ccols):
            om[:, ccols] = cnt_lists[m]
        out[:, bs, :] = om.reshape(S, BLOC, H)
    return out


# revision 10
# speedup vs baseline: 1.3782x; 1.0795x over previous
"""Trainium2 Bass kernel for nn_CategoricalActivation (8-core data-parallel).

Reference semantics (per element x[s, b, h], column col=(b, h)):
    ss = x / (1 + |x|)                            # softsign
    boundaries b_c = x_raw[ind[c, col], col]      # 4 sampled rows per column
    counts = #{c : x > b_c} - 2.5
    cat  = cat_u[col] < 0.1
    ord  = (ord_u[col] < 0.7) & cat
    out  = ord ? 0.0 : (cat ? counts : ss)
(The "randomize_classes" remap is identically zero: counts values
{-2.5..1.5} never equal a class id 0..4, so remapped == 0 at ord cols.)

v7 design (per core):
  - Device does the bulk softsign stream; everything per-column/sparse
    (boundary gathers, counts for the ~3% catno columns, ord zeros,
    scatter) happens on the host while staging/unsharding.  Rationale
    from v6 tracing: side-channel tensors span only <=68 SBUF partitions,
    so their DMA descriptors pile onto a few DMA engines; the last bulk
    store's completion then trails the slowest engine by ~6.5us.  A pure
    [128, *] stream keeps all 16 engines perfectly balanced.
  - ALL categorical columns (~10%) are compacted OUT of the bulk on the
    host: bulk is [S, KEEP=1872] bf16 (non-cat columns, padded), staged
    in device chunk order [TCH=16, 128, 1872] so each chunk DMA is one
    contiguous 479 KB block.
  - Chunk DMAs stay ~0.5 MB: DMAs outstanding on the HWDGE ring progress
    CONCURRENTLY (fair packet-level round-robin, not FIFO), so a few
    large upfront loads all complete clustered at the read-stream end
    and compute starts ~20us late (v4/v5 lost 8-15us to this).  With 16
    chunk loads dispatched back-to-back the first chunk lands ~3us after
    dispatch and the softsign pipeline (DVE |x| -> ACT 1/(1+|x|) -> DVE
    mult -> store) runs just behind the read stream.
  - The first out-DMA is gated on the LAST in-DMA (extra semaphore wait
    patched in after scheduling): the ring then does one pure-read burst
    followed by one pure-write burst instead of packet-interleaving
    reads with writes mid-stream (HBM bus turnaround costs bandwidth).
"""

import numpy as np

S = 2048
B = 16
H = 1024
NCORES = 8
BLOC = B // NCORES         # 2
C = BLOC * H               # 2048 columns per core
P = 128
KEEP = 1872                # padded non-cat (bulk) column slots per core
TCH = S // P               # 16 row chunks
NC5 = 5
GATE_OUTS = True           # first store waits for a late load (burst phases)
GATE_IN = 10               # gate on this in-chunk: the last reads drain while
                           # the first write's descriptors+receipt spin up
                           # (v8 gated on 13: still a 4.6us idle bubble at the
                           # read->write transition - sem receipt is ~2-3us)
OG = 2                     # chunks per out-DMA (big stores: write burst is
                           # HBM-paced, not SP-dispatch-paced)

_CACHE = {}


def _split_multi_waits(nc, scr_ap=None, max_waits=1):
    """This container's walrus rejects >1 sync-wait per instruction; hoist
    extra waits onto cheap same-engine carrier instructions inserted just
    before (tiny Memset on the pipelined engines - a Drain there would
    flush the pipe at ~0.4-2.4us - and Drain on the sequencer-only ones)."""
    import concourse.mybir as mybir

    memset_engines = {mybir.EngineType.DVE, mybir.EngineType.Pool}
    n_split = 0
    for f in nc.m.functions:
        for blk in f.blocks:
            insts = blk.instructions
            i = 0
            while i < len(insts):
                ins = insts[i]
                si = ins.sync_info
                if si is not None and len(si.on_wait) > max_waits:
                    waits = list(si.on_wait)
                    keep = waits[-max_waits:]
                    hoist = waits[:-max_waits]
                    for w in hoist:
                        if scr_ap is not None and ins.engine in memset_engines:
                            d = mybir.InstMemset(
                                name=f"I-{nc.next_id()}", mode="Const",
                                ins=[], outs=[scr_ap], constant=0)
                        else:
                            d = mybir.InstDrain(
                                name=f"I-{nc.next_id()}", ins=[], outs=[],
                                bass_is_fusable=False)
                        d.engine = ins.engine
                        d.sync_info = mybir.SyncInfo(on_wait=[w], on_update=[])
                        insts.insert(i, d)
                        i += 1
                        n_split += 1
                    si.on_wait = keep
                    ins.sync_info = si
                i += 1
    return n_split


def _gate_outs_on_last_in(nc):
    """Insert one SP Drain before the first out-DMA waiting on the last
    in-DMA's completion semaphore: the ring finishes the read burst before
    any write descriptors queue behind it (avoids HBM read/write
    interleaving mid-stream).  Post-scheduling BIR patch."""
    import concourse.mybir as mybir

    for f in nc.m.functions:
        for blk in f.blocks:
            insts = blk.instructions
            last_in = None           # (sem_id, cumulative threshold)
            cum = {}
            n_in = 0
            first_out_idx = None
            for i, ins in enumerate(insts):
                if not isinstance(ins, mybir.InstDMACopy):
                    continue
                si = ins.sync_info
                upd = si.on_update[0] if si and si.on_update else None
                if upd is not None:
                    cum[upd.id] = cum.get(upd.id, 0) + upd.update_value
                src = ins.ins[0].memref if ins.ins else ""
                dst = ins.outs[0].memref if ins.outs else ""
                if src == "x" and upd is not None:
                    if n_in <= GATE_IN:
                        last_in = (upd.id, cum[upd.id], upd.ant_name)
                    n_in += 1
                if dst == "out" and first_out_idx is None:
                    first_out_idx = i
            if last_in is None or first_out_idx is None:
                continue
            sem_id, thresh, ant = last_in
            w = mybir.SyncWait(sync_type="semaphore", id=sem_id,
                               ant_name=ant, wait_mode="sem-ge-imm",
                               wait_value=thresh)
            d = mybir.InstDrain(name=f"I-{nc.next_id()}", ins=[], outs=[],
                                bass_is_fusable=False)
            d.engine = insts[first_out_idx].engine
            d.sync_info = mybir.SyncInfo(on_wait=[w], on_update=[])
            insts.insert(first_out_idx, d)
            return True
    return False


def _act_unary(nc, out_ap, in_ap, func, bias=0.0):
    """One scalar-engine activation, float-immediate bias (bypasses the
    bass wrapper so Reciprocal is allowed; HW-measured ~1.2e-5 max err)."""
    import concourse.mybir as mybir

    eng = nc.scalar
    ins_ = [
        eng.lower_ap(in_ap),
        mybir.ImmediateValue(dtype=mybir.dt.float32, value=float(bias)),
        mybir.ImmediateValue(dtype=mybir.dt.float32, value=1.0),
        mybir.ImmediateValue(dtype=mybir.dt.float32, value=0.0),
    ]
    return eng.add_instruction(
        mybir.InstActivation(
            name=nc.get_next_instruction_name(),
            func=func,
            ins=ins_,
            outs=[eng.lower_ap(out_ap)],
        )
    )


def _build_program():
    import contextlib

    import concourse.bass as bass
    import concourse.tile as tile
    from concourse import mybir

    A = mybir.AluOpType
    F = mybir.ActivationFunctionType
    bf16 = mybir.dt.bfloat16
    i16 = mybir.dt.int16
    i32 = mybir.dt.int32

    nc = bass.Bass()
    x_in = nc.dram_tensor("x", [TCH, P, KEEP], bf16, kind="ExternalInput")
    out_d = nc.dram_tensor("out", [TCH // OG, P, OG * KEEP], bf16,
                           kind="ExternalOutput")

    with tile.TileContext(nc) as tc:
        with contextlib.ExitStack() as ctx:
            singles = ctx.enter_context(tc.tile_pool(name="singles", bufs=1))
            xp = ctx.enter_context(tc.tile_pool(name="xp", bufs=TCH))
            up = ctx.enter_context(tc.tile_pool(name="up", bufs=6))
            po = ctx.enter_context(tc.tile_pool(name="po", bufs=TCH // OG))

            scr = singles.tile([1, 8], i32, name="scr")
            nc.vector.memset(scr, 0)

            # every chunk in-DMA upfront, one SBUF slot each: loads never
            # wait on buffer recycling; chunk 0 completes ~3us after
            # dispatch (ring fair-share stays shallow early)
            xts = []
            for t in range(TCH):
                xt = xp.tile([P, KEEP], bf16, tag="xt", name=f"xt{t}")
                nc.sync.dma_start(out=xt, in_=x_in[t, :, :])
                xts.append(xt)

            ot = None
            for t in range(TCH):
                g, h = divmod(t, OG)
                xt = xts[t]
                absx = up.tile([P, KEEP], bf16, tag="absx", name="absx")
                nc.vector.tensor_scalar(out=absx.bitcast(i16),
                                        in0=xt.bitcast(i16),
                                        scalar1=0x7FFF, scalar2=None,
                                        op0=A.bitwise_and)
                ract = up.tile([P, KEEP], bf16, tag="ract", name="ract")
                _act_unary(nc, ract[:, :], absx[:, :], F.Reciprocal, bias=1.0)
                if h == 0:
                    ot = po.tile([P, OG * KEEP], bf16, tag="ot", name=f"ot{g}")
                nc.vector.tensor_tensor(out=ot[:, h * KEEP:(h + 1) * KEEP],
                                        in0=xt, in1=ract, op=A.mult)
                if h == OG - 1:
                    nc.sync.dma_start(out=out_d[g, :, :], in_=ot)

    _split_multi_waits(nc, scr_ap=nc.vector.lower_ap(scr[0:1, 0:1]))
    if GATE_OUTS:
        _gate_outs_on_last_in(nc)
    return nc


def _stage_bulk(xk):
    """[S, KEEP] f32 -> device chunk order [TCH, P, KEEP] bf16."""
    import ml_dtypes
    return np.ascontiguousarray(
        xk.reshape(TCH, P, KEEP)).astype(ml_dtypes.bfloat16)


def _unstage_bulk(ob):
    """[TCH//OG, P, OG*KEEP] bf16 -> [S, KEEP] f32."""
    v = np.asarray(ob).astype(np.float32)
    return v.reshape(TCH // OG, P, OG, KEEP).transpose(0, 2, 1, 3).reshape(S, KEEP)


def kernel(x, ind, cat_u, ord_u, perm, num_classes):
    from concourse.bass_utils import run_bass_kernel_spmd

    assert int(num_classes) == NC5
    x = np.ascontiguousarray(x, dtype=np.float32)
    ind = np.ascontiguousarray(ind, dtype=np.int32)
    cat_u = np.asarray(cat_u, dtype=np.float32)
    ord_u = np.asarray(ord_u, dtype=np.float32)
    assert x.shape == (S, B, H) and ind.shape == (4, B, H)

    cat = cat_u < np.float32(0.1)
    ordm = (ord_u < np.float32(0.7)) & cat
    catno = cat & ~ordm
    in_maps = []
    keep_lists = []
    cat_lists = []
    cnt_lists = []
    for m in range(NCORES):
        bs = slice(BLOC * m, BLOC * (m + 1))
        xm = x[:, bs, :].reshape(S, C)
        indm = ind[:, bs, :].reshape(4, C)
        kcols = np.nonzero(~cat[bs].reshape(C))[0].astype(np.int32)
        ccols = np.nonzero(catno[bs].reshape(C))[0].astype(np.int32)
        nk = len(kcols)
        assert nk <= KEEP, f"core {m}: {nk} keep columns exceed KEEP"
        keep_lists.append(kcols)
        cat_lists.append(ccols)
        xk = np.zeros((S, KEEP), np.float32)
        xk[:, :nk] = xm[:, kcols]
        # counts for the ~3% catno columns: f32 compares, exactly the
        # reference ordering (softsign is strictly monotone)
        v = xm[:, ccols]                          # [S, kc]
        t_ = xm[indm[:, ccols], ccols]            # [4, kc]
        cnt_lists.append((v[None] > t_[:, None]).sum(0).astype(np.float32)
                         - np.float32(2.5))      # [S, kc]
        in_maps.append({"x": _stage_bulk(xk)})

    if "nc" not in _CACHE:
        _CACHE["nc"] = _build_program()
    res = run_bass_kernel_spmd(_CACHE["nc"], in_maps,
                               core_ids=list(range(NCORES)))
    out = np.empty((S, B, H), np.float32)
    for m in range(NCORES):
        bs = slice(BLOC * m, BLOC * (m + 1))
        om = np.zeros((S, C), np.float32)
        kcols, ccols = keep_lists[m], cat_lists[m]
        ok = _unstage_bulk(res.results[m]["out"])
        om[:, kcols] = ok[:, :len(kcols)]
        if len(ccols):
            om[:, ccols] = cnt_lists[m]
        out[:, bs, :] = om.reshape(S, BLOC, H)
    return out
